# revision 17
# baseline (speedup 1.0000x reference)
"""Trainium2 Bass kernel for nn_CustomMLPLayer_13408887898971 (topk_masking).

Computes (matching reference.py):
    scores = sum_s relu(x[0,s,:])          # [d_ff]
    idx    = top_k(scores, K)              # K = 4403
    out    = x[..., idx] @ W[:, idx].T     # [1, S, d_model]

Strategy (8 NeuronCores, tensor-parallel over d_model), sparse mode:
  - host: transpose x and W to j-major (contraction on partitions),
    shard W.T by d_model columns (512 per core); x.T replicated (bf16
    for the GEMM, f32 token-shard for the exact score reduction).
  - device, per core:
      A: partial scores over this core's 256-token shard with a two-limb
         (integer part + fractional residue, scaled by 1024) split so the
         cross-core sum is exact to ~fp64 (the top-K boundary gap is only
         a few f32 ULP).
      B: AllReduce partial scores across the 8 cores (88KB)
      C: exact K-th largest via radix-16 search on the f32 bit pattern
      C2: compacted index list of the selected j's via gpsimd
          sparse_gather (stream compaction), replicated into the
          16-partition-wrapped int16 layout dma_gather wants
      D: dma_gather the K (padded to 4480) selected rows of x.T and W.T
         from HBM into SBUF (bf16) and run the compacted dense GEMM
         x_sel.T @ W_sel -> out.T shard; contraction is 4480 instead of
         11008 and runs at bf16 PE rate (1 cy/row) instead of fp32.
  - host: concat per-core [512, 2048] out.T shards, transpose.
"""

import numpy as np

N_CORES = 8

FULL_CFG = dict(
    dff=11008,
    s=2048,
    d=4096,
    k=4403,
    name="sparse",
    mode="sparse",
)

DENSE_CFG = dict(
    dff=11008,
    s=2048,
    d=4096,
    k=4403,
    name="dense",
    mode="dense",
    mm_dtype="bf16",
)

_cache = {}


def _radix_select_threshold(nc, tc, pp, psum_red, ones128, bass_isa, mybir, scores, K, JT):
    """Exact K-th largest score via radix-16 search on the f32 bit pattern.

    scores: [128, JT] f32, non-negative. Returns thr_f [128, 1] f32 tile
    (same value in every partition) with count(scores >= thr_f) >= K and
    count(scores >= next ulp) < K.

    The DVE ALU evaluates int32 tensor ops in f32 arithmetic, so bit-space
    increments below ULP(thr_bits ~ 2^30) = 128 round away. The int-bit
    stage resolves bits 7..30 (increments are multiples of 128 -> exact in
    f32); the low 7 bits are resolved in float space with exact ULP steps.

    The per-round cross-partition count reduction runs on the PE (ones
    matmul into `psum_red`: every output partition holds the column sum)
    instead of gpsimd partition_all_reduce - far lower latency. Candidate
    vectors are built 15-at-a-time in single DVE ops.
    """
    dt = mybir.dt
    Alu = mybir.AluOpType

    thr = pp.tile([128, 1], dt.int32, tag="thr")
    thrv = pp.tile([128, 1], dt.float32, tag="thrv")
    cands = pp.tile([128, 15], dt.int32, tag="cands")
    candf = pp.tile([128, 15], dt.float32, tag="candf")
    ge_scr = pp.tile([128, JT], dt.float32, tag="ge_scr")
    cnts = pp.tile([128, 15], dt.float32, tag="cnts")
    sel = pp.tile([128, 15], dt.float32, tag="sel")
    digf = pp.tile([128, 1], dt.float32, tag="digf")
    digi = pp.tile([128, 1], dt.int32, tag="digi")
    thr_f = pp.tile([128, 1], dt.float32, tag="thr_f")
    ulp = pp.tile([128, 1], dt.float32, tag="ulp")
    ulpm = pp.tile([128, 1], dt.float32, tag="ulpm")
    step = pp.tile([128, 1], dt.float32, tag="step")
    # rowi[:, r-1] = r (int32, every partition); rowv = same as f32
    rowi = pp.tile([128, 15], dt.int32, tag="rowi")
    rowv = pp.tile([128, 15], dt.float32, tag="rowv")
    nc.gpsimd.iota(rowi[:], pattern=[[1, 15]], base=1, channel_multiplier=0)
    nc.vector.tensor_copy(out=rowv[:], in_=rowi[:])

    nc.vector.memset(thrv[:], 0)

    def count_round(ncand, upd):
        """Counts candf[:, 0..ncand-1], picks the digit into digf, updates."""
        for r in range(ncand):
            nc.vector.tensor_scalar(
                out=ge_scr[:],
                in0=scores[:],
                scalar1=candf[:, r : r + 1],
                scalar2=0.0,
                op0=Alu.is_ge,
                op1=Alu.add,
                accum_out=cnts[:, r : r + 1],
            )
        nc.tensor.matmul(
            psum_red[:, :ncand],
            lhsT=ones128[:],
            rhs=cnts[:, :ncand],
            start=True,
            stop=True,
        )
        nc.vector.tensor_scalar(
            out=sel[:, :ncand],
            in0=psum_red[:, :ncand],
            scalar1=float(K),
            scalar2=0.0,
            op0=Alu.is_ge,
            op1=Alu.add,
            accum_out=digf[:],
        )
        upd()

    # --- int-bit stage: bits 7..30, radix 16 ---
    # The threshold bit pattern is carried as an f32 NUMERIC value thrv
    # (exact: always a multiple of 2^7 below 2^31); cands casts it to the
    # int32 bit pattern for the bitcast-compare.
    for shift in (27, 23, 19, 15, 11, 7):
        # cands[:, r-1] = thrv + (r << shift)
        nc.vector.tensor_scalar(
            out=cands[:],
            in0=rowi[:],
            scalar1=float(1 << shift),
            scalar2=thrv[:],
            op0=Alu.mult,
            op1=Alu.add,
        )
        nc.vector.tensor_scalar(
            out=candf[:],
            in0=cands[:].bitcast(dt.float32),
            scalar1=0.0,
            scalar2=None,
            op0=Alu.add,
        )

        def upd_int(shift=shift):
            nc.vector.tensor_scalar(
                out=thrv[:],
                in0=digf[:],
                scalar1=float(1 << shift),
                scalar2=thrv[:],
                op0=Alu.mult,
                op1=Alu.add,
            )

        count_round(15, upd_int)

    nc.vector.tensor_scalar(
        out=thr[:], in0=thrv[:], scalar1=0.0, scalar2=None, op0=Alu.add
    )

    # --- float stage: low 7 bits with exact ULP steps ---
    nc.vector.tensor_scalar(
        out=digi[:], in0=thr[:], scalar1=128, scalar2=None, op0=Alu.add
    )
    nc.vector.tensor_tensor(
        out=ulp[:],
        in0=digi[:].bitcast(dt.float32),
        in1=thr[:].bitcast(dt.float32),
        op=Alu.subtract,
    )
    nc.vector.tensor_scalar(
        out=ulp[:], in0=ulp[:], scalar1=1.0 / 128.0, scalar2=None, op0=Alu.mult
    )
    nc.vector.tensor_scalar(
        out=thr_f[:],
        in0=thr[:].bitcast(dt.float32),
        scalar1=0.0,
        scalar2=None,
        op0=Alu.add,
    )

    for mult_, ncand in ((16, 7), (1, 15)):
        # candf[:, r-1] = thr_f + (r * mult_) * ulp
        nc.vector.tensor_scalar(
            out=ulpm[:],
            in0=ulp[:],
            scalar1=float(mult_),
            scalar2=None,
            op0=Alu.mult,
        )
        nc.vector.tensor_scalar(
            out=candf[:, :ncand],
            in0=rowv[:, :ncand],
            scalar1=ulpm[:],
            scalar2=thr_f[:],
            op0=Alu.mult,
            op1=Alu.add,
        )

        def upd_f(mult_=mult_):
            nc.vector.tensor_scalar(
                out=digf[:],
                in0=digf[:],
                scalar1=float(mult_),
                scalar2=None,
                op0=Alu.mult,
            )
            nc.vector.tensor_tensor(out=step[:], in0=digf[:], in1=ulp[:], op=Alu.mult)
            nc.vector.tensor_tensor(
                out=thr_f[:], in0=thr_f[:], in1=step[:], op=Alu.add
            )

        count_round(ncand, upd_f)

    return thr_f


def _build_sparse(cfg):
    """Sparse top-K gather + compacted bf16 GEMM program."""
    from concourse import bacc, tile
    import concourse.bass as bass
    import concourse.mybir as mybir
    import concourse.bass_isa as bass_isa

    dt = mybir.dt
    Alu = mybir.AluOpType
    Act = mybir.ActivationFunctionType

    DFF = cfg["dff"]          # 11008
    S = cfg["s"]              # 2048
    D = cfg["d"]              # 4096
    K = cfg["k"]              # 4403
    DSH = D // N_CORES        # 512
    SSH = S // N_CORES        # 256
    JT = DFF // 128           # 86
    NKT = 35                  # k tiles (4480 slots >= K)
    NSLOT = NKT * 128         # 4480
    IDXW = NSLOT // 16        # 280 idx columns (16-wrapped)
    NXCH = 5                  # gather chunks
    KTC = NKT // NXCH         # 7 k-tiles per chunk
    CIDX = KTC * 128 // 16    # 56 idx cols per chunk
    CSLOT = KTC * 128         # 896 slots per chunk
    SGW = DFF // 16           # 688 sparse-gather input cols
    SCH = 512                 # matmul moving dim
    NCH = S // SCH            # 4 s-chunks
    DT_ = DSH // 128          # 4 d tiles

    nc = bacc.Bacc(
        "TRN2", target_bir_lowering=False, debug=False, num_devices=N_CORES
    )

    xs = nc.dram_tensor("xs", [DFF, SSH], dt.float32, kind="ExternalInput").ap()
    xt = nc.dram_tensor("xt", [DFF, S], dt.bfloat16, kind="ExternalInput").ap()
    wt = nc.dram_tensor("wt", [DFF, DSH], dt.bfloat16, kind="ExternalInput").ap()
    outT = nc.dram_tensor("outT", [DSH, S], dt.float32, kind="ExternalOutput").ap()

    with tile.TileContext(nc) as tc:
        with (
            tc.tile_pool(name="persist", bufs=1) as pp,
            tc.tile_pool(name="xs_p", bufs=3) as xsp,
            tc.tile_pool(name="relu_p", bufs=2) as rlp,
            tc.tile_pool(name="out_p", bufs=2) as otp,
            tc.tile_pool(name="psum", bufs=1, space="PSUM") as psp,
            tc.tile_pool(name="dram", bufs=1, space="DRAM") as drp,
        ):
            # Q7 scratch limits sparse_gather to in+out <= ~44KB, i.e.
            # [16, 344] -> [16, 344] per call. Compact the 11008 slots in two
            # 5504-slot halves, then concatenate the two lists in DRAM: list
            # A at offset 0, list B at element offset nA (register-offset
            # DMA). Ascending writes make A's junk tail disappear under B's
            # copy, and B's tail is masked to -1 so slots [K', 4480) read
            # back as -1 (ignored by dma_gather).
            HT = JT // 2              # 43 j-tiles per half
            SGH = 16 * HT * 8 // 16   # 344 = free cols per half ([16, 344])
            HSLOT = 16 * SGH          # 5504 slots per half

            partial = pp.tile([128, 2 * JT], dt.float32, tag="partial")
            scores = pp.tile([128, JT], dt.float32, tag="scores")
            sc16 = [
                pp.tile([16, SGH], dt.float32, tag=f"sc16_{h}", name=f"sc16_{h}")
                for h in range(2)
            ]
            iotaf1 = [
                pp.tile([16, SGH], dt.float32, tag=f"iotaf1_{h}", name=f"iotaf1_{h}")
                for h in range(2)
            ]
            ioti = pp.tile([16, SGH], dt.int32, tag="ioti")
            selj = pp.tile([16, SGH], dt.float32, tag="selj")
            sgout = [
                pp.tile([16, SGH], dt.float32, tag=f"sgout_{h}", name=f"sgout_{h}")
                for h in range(2)
            ]
            nf = [
                pp.tile([1, 1], dt.uint32, tag=f"nf_{h}", name=f"nf_{h}")
                for h in range(2)
            ]
            slot16 = pp.tile([16, SGH], dt.float32, tag="slot16")
            nb16 = pp.tile([16, 1], dt.uint32, tag="nb16")
            nb16f = pp.tile([16, 1], dt.float32, tag="nb16f")
            idxsf = pp.tile([16, IDXW], dt.float32, tag="idxsf")
            idxs16 = pp.tile([128, IDXW], dt.int16, tag="idxs16")
            xg = pp.tile([128, NKT, S], dt.bfloat16, tag="xg")
            wg = pp.tile([128, NKT, DSH], dt.bfloat16, tag="wg")

            # ---- independent setup (scheduler runs these during phase A) --
            # Half h covers j in [5504h, 5504(h+1)); slot (p, f=a*43+b) of
            # half h holds j = 5504h + 16a + 128b + p, matching the 8
            # per-16-partition-block column copies below.
            for h in range(2):
                nc.gpsimd.iota(
                    ioti[:],
                    pattern=[[16, 8], [128, HT]],
                    base=HSLOT * h + 1,
                    channel_multiplier=1,
                )
                nc.vector.tensor_copy(out=iotaf1[h][:], in_=ioti[:])
            # slot-number table for tail masking: slot16[p, f] = 16f + p
            nc.gpsimd.iota(
                ioti[:], pattern=[[16, SGH]], base=0, channel_multiplier=1
            )
            nc.vector.tensor_copy(out=slot16[:], in_=ioti[:])
            # pad slots (>= K') of the last k-tile stay zero => zero
            # contribution in the GEMM, no masking needed anywhere.
            nc.vector.memset(xg[:, NKT - 1 : NKT, :], 0)
            nc.vector.memset(wg[:, NKT - 1 : NKT, :], 0)
            ones128 = pp.tile([128, 128], dt.float32, tag="ones128")
            nc.vector.memset(ones128[:], 1.0)

            # ---- phase A: partial scores over this core's token shard ----
            # Two-limb trick on r = relu(x)*1024: integer part h sums EXACTLY
            # in f32 (partials are integers < 2^24); residue |r1| <= 0.5 sums
            # with ~1e-6 relative noise. h = (r + 2^23) - 2^23 (RNE round).
            for t in range(JT):
                st = xsp.tile([128, SSH], dt.float32)
                nc.sync.dma_start(st[:], xs[t * 128 : (t + 1) * 128, :])
                rt = rlp.tile([128, SSH], dt.float32, tag="rt")
                nc.scalar.activation(rt[:], st[:], Act.Relu, scale=1024.0)
                tmpt = rlp.tile([128, SSH], dt.float32, tag="tmpt")
                nc.vector.tensor_scalar(
                    out=tmpt[:],
                    in0=rt[:],
                    scalar1=float(2.0**23),
                    scalar2=None,
                    op0=Alu.add,
                )
                ht = rlp.tile([128, SSH], dt.float32, tag="ht")
                nc.vector.tensor_scalar(
                    out=ht[:],
                    in0=tmpt[:],
                    scalar1=float(2.0**23),
                    scalar2=0.0,
                    op0=Alu.subtract,
                    op1=Alu.add,
                    accum_out=partial[:, t : t + 1],
                )
                r1t = rlp.tile([128, SSH], dt.float32, tag="r1t")
                # r1 on the (otherwise idle) gpsimd engine; DVE keeps the
                # accumulating ops it alone supports.
                nc.gpsimd.tensor_tensor(
                    out=r1t[:], in0=rt[:], in1=ht[:], op=Alu.subtract
                )
                nc.vector.tensor_reduce(
                    out=partial[:, JT + t : JT + t + 1],
                    in_=r1t[:],
                    axis=mybir.AxisListType.X,
                    op=Alu.add,
                )

            # ---- phase B: AllReduce partial sums across cores ----
            cc_in = drp.tile([128, 2 * JT], dt.float32)
            cc_out = drp.tile([128, 2 * JT], dt.float32)
            nc.sync.dma_start(cc_in[:], partial[:])
            nc.gpsimd.collective_compute(
                "AllReduce",
                Alu.add,
                replica_groups=[list(range(N_CORES))],
                ins=[cc_in.opt()],
                outs=[cc_out.opt()],
            )
            hr = pp.tile([128, 2 * JT], dt.float32, tag="hr")
            nc.sync.dma_start(hr[:], cc_out[:])
            nc.vector.tensor_tensor(
                out=scores[:], in0=hr[:, :JT], in1=hr[:, JT:], op=Alu.add
            )
            nc.vector.tensor_scalar(
                out=scores[:],
                in0=scores[:],
                scalar1=float(2.0**-10),
                scalar2=None,
                op0=Alu.mult,
            )

            # ---- phase C: K-th largest threshold ----
            psC = psp.tile([128, SCH], dt.float32, tag="ps0", name="psC")
            thr_f = _radix_select_threshold(
                nc, tc, pp, psC[:], ones128[:], bass_isa, mybir, scores, K, JT
            )

            # ---- phase C2: compacted index list (two halves) ----
            # sc16[h][p, 43a + t] = scores[16a + p, 43h + t], i.e. slot
            # (p, f) of half h holds the score of j = 5504h + 16a + 128t + p,
            # matching iotaf1[h] (which holds j+1).
            for h in range(2):
                for a in range(8):
                    nc.sync.dma_start(
                        sc16[h][:, HT * a : HT * (a + 1)],
                        scores[16 * a : 16 * (a + 1), HT * h : HT * (h + 1)],
                    )

            cat = drp.tile([1, 2 * HSLOT], dt.float32)

            for h in range(2):
                # selj = selected ? j : -1  ==  (sc16 >= thr) * (j + 1) - 1
                nc.vector.tensor_scalar(
                    out=selj[:],
                    in0=sc16[h][:],
                    scalar1=thr_f[0:16, :],
                    scalar2=None,
                    op0=Alu.is_ge,
                )
                nc.vector.tensor_tensor(
                    out=selj[:], in0=selj[:], in1=iotaf1[h][:], op=Alu.mult
                )
                nc.vector.tensor_scalar(
                    out=selj[:],
                    in0=selj[:],
                    scalar1=1.0,
                    scalar2=None,
                    op0=Alu.subtract,
                )
                nc.gpsimd.sparse_gather(sgout[h][:], selj[:], num_found=nf[h][:])

            # mask list B's junk tail (hw writes arbitrary values past
            # num_found): slots >= nB -> -1.
            nc.vector.tensor_copy(out=nb16[0:1, :], in_=nf[1][:])
            for p in (1, 2, 4, 8):
                nc.sync.dma_start(nb16[p : 2 * p, :], nb16[0:p, :])
            nc.vector.tensor_copy(out=nb16f[:], in_=nb16[:])
            bmask = selj  # reuse scratch
            nc.vector.tensor_scalar(
                out=bmask[:],
                in0=slot16[:],
                scalar1=nb16f[:],
                scalar2=None,
                op0=Alu.is_lt,
            )
            # sgoutB = valid ? sgoutB : -1 == bmask * (sgoutB + 1) - 1
            nc.vector.tensor_scalar(
                out=sgout[1][:],
                in0=sgout[1][:],
                scalar1=1.0,
                scalar2=None,
                op0=Alu.add,
            )
            nc.vector.tensor_tensor(
                out=sgout[1][:], in0=sgout[1][:], in1=bmask[:], op=Alu.mult
            )
            nc.vector.tensor_scalar(
                out=sgout[1][:],
                in0=sgout[1][:],
                scalar1=1.0,
                scalar2=None,
                op0=Alu.subtract,
            )

            # registers (Pool engine: same queue as the gpsimd DMAs below)
            nAr = nc.alloc_register(mybir.EngineType.Pool, "nAr")
            nfr = nc.alloc_register(mybir.EngineType.Pool, "nfr")
            wreg = nc.alloc_register(mybir.EngineType.Pool, "wreg")
            r4 = nc.alloc_register(mybir.EngineType.Pool, "r4")
            nc.gpsimd.reg_load(nAr, nf[0][:1, :1])
            nc.gpsimd.reg_load(nfr, nf[1][:1, :1])
            nc.gpsimd.reg_alu(nfr, nfr, nAr, Alu.add)
            nc.gpsimd.reg_alu(wreg, nfr, NSLOT, Alu.min)
            nc.gpsimd.reg_alu(r4, wreg, (NXCH - 1) * CSLOT, Alu.subtract)

            # concatenate in DRAM (linear slot order; de-wrap [16, F] via
            # [(1,16),(16,F)] APs): A at 0, B at element offset nA.
            cat_t = cat[:].tensor
            catA = bass.AP(cat_t, 0, [[1, 16], [16, SGH]])
            nc.gpsimd.dma_start(out=catA, in_=sgout[0][:])
            catB = bass.AP(
                cat_t, nAr, [[1, 16], [16, SGH]], dep_tracking_offset=0
            )
            nc.gpsimd.dma_start(out=catB, in_=sgout[1][:])
            # read back the first 4480 slots, re-wrapped to [16, 280]
            catR = bass.AP(cat_t, 0, [[1, 16], [16, IDXW]])
            nc.sync.dma_start(out=idxsf[:], in_=catR)
            nc.vector.tensor_copy(out=idxs16[0:16, :], in_=idxsf[:])
            for b in range(1, 8):
                nc.sync.dma_start(idxs16[16 * b : 16 * (b + 1), :], idxs16[0:16, :])

            # ---- phase D: gathers + compacted GEMM ----
            # All W chunks first (small), then X: once the first X chunk
            # lands the remaining ones always arrive faster than the PE
            # consumes them, so the matmul stream never stalls (stalls
            # would also drop the PE back to the slow p-state).
            for i in range(NXCH):
                ri = CSLOT if i < NXCH - 1 else r4
                nc.gpsimd.dma_gather(
                    out_ap=wg[:, KTC * i : KTC * (i + 1), :],
                    in_ap=wt[:],
                    idxs_ap=idxs16[:, CIDX * i : CIDX * (i + 1)],
                    num_idxs=CSLOT,
                    num_idxs_reg=ri,
                    elem_size=DSH,
                )
            for i in range(NXCH):
                ri = CSLOT if i < NXCH - 1 else r4
                nc.gpsimd.dma_gather(
                    out_ap=xg[:, KTC * i : KTC * (i + 1), :],
                    in_ap=xt[:],
                    idxs_ap=idxs16[:, CIDX * i : CIDX * (i + 1)],
                    num_idxs=CSLOT,
                    num_idxs_reg=ri,
                    elem_size=S,
                )

            for half in range(2):
                cs = (2 * half, 2 * half + 1)
                psums = [
                    psp.tile([128, SCH], dt.float32, tag=f"ps{j}", name=f"ps_h{half}_{j}")
                    for j in range(8)
                ]
                for g in range(NKT):
                    for ci, c in enumerate(cs):
                        rhs = xg[:, g : g + 1, c * SCH : (c + 1) * SCH].squeeze(1)
                        for d in range(DT_):
                            nc.tensor.matmul(
                                psums[ci * DT_ + d][:],
                                lhsT=wg[
                                    :, g : g + 1, d * 128 : (d + 1) * 128
                                ].squeeze(1),
                                rhs=rhs,
                                start=(g == 0),
                                stop=(g == NKT - 1),
                            )
                for ci, c in enumerate(cs):
                    for d in range(DT_):
                        ot = otp.tile([128, SCH], dt.float32)
                        nc.scalar.copy(ot[:], psums[ci * DT_ + d][:])
                        nc.sync.dma_start(
                            outT[d * 128 : (d + 1) * 128, c * SCH : (c + 1) * SCH],
                            ot[:],
                        )

    nc.compile()
    return nc


def _build_dense(cfg):
    """Masked dense GEMM program (fallback / ablation)."""
    from concourse import bacc, tile
    import concourse.bass as bass
    import concourse.mybir as mybir
    import concourse.bass_isa as bass_isa

    dt = mybir.dt
    Alu = mybir.AluOpType
    Act = mybir.ActivationFunctionType

    DFF = cfg["dff"]
    S = cfg["s"]
    D = cfg["d"]
    K = cfg["k"]
    DSH = D // N_CORES
    SSH = S // N_CORES
    JT = DFF // 128
    SCH = min(512, S)
    NSCH = S // SCH
    DT_ = max(1, DSH // 128)
    DW = min(128, DSH)

    mmdt = dt.float32 if cfg.get("mm_dtype", "bf16") == "f32" else dt.bfloat16

    nc = bacc.Bacc(
        "TRN2", target_bir_lowering=False, debug=False, num_devices=N_CORES
    )

    xs = nc.dram_tensor("xs", [DFF, SSH], dt.float32, kind="ExternalInput").ap()
    xt = nc.dram_tensor("xt", [DFF, S], mmdt, kind="ExternalInput").ap()
    wt = nc.dram_tensor("wt", [DFF, DSH], mmdt, kind="ExternalInput").ap()
    outT = nc.dram_tensor("outT", [DSH, S], dt.float32, kind="ExternalOutput").ap()

    with tile.TileContext(nc) as tc:
        with (
            tc.tile_pool(name="persist", bufs=1) as pp,
            tc.tile_pool(name="xs_p", bufs=3) as xsp,
            tc.tile_pool(name="relu_p", bufs=3) as rlp,
            tc.tile_pool(name="xt_p", bufs=4) as xtp,
            tc.tile_pool(name="wt_p", bufs=4) as wtp,
            tc.tile_pool(name="wm_p", bufs=4) as wmp,
            tc.tile_pool(name="out_p", bufs=3) as otp,
            tc.tile_pool(name="psum", bufs=2, space="PSUM") as psp,
            tc.tile_pool(name="dram", bufs=1, space="DRAM") as drp,
        ):
            partial = pp.tile([128, 2 * JT], dt.float32, tag="partial")
            scores = pp.tile([128, JT], dt.float32, tag="scores")
            mask = pp.tile([128, JT], dt.float32, tag="mask")
            c23 = pp.tile([128, 1], dt.float32, tag="c23")
            nc.vector.memset(c23[:], float(2.0**23))
            ones128 = pp.tile([128, 128], dt.float32, tag="ones128")
            nc.vector.memset(ones128[:], 1.0)

            for t in range(JT):
                st = xsp.tile([128, SSH], dt.float32)
                nc.sync.dma_start(st[:], xs[t * 128 : (t + 1) * 128, :])
                rt = rlp.tile([128, SSH], dt.float32, tag="rt")
                nc.scalar.activation(rt[:], st[:], Act.Relu, scale=1024.0)
                tt_ = rlp.tile([128, SSH], dt.float32, tag="tt")
                nc.scalar.activation(tt_[:], rt[:], Act.Identity, bias=c23[:])
                ht = rlp.tile([128, SSH], dt.float32, tag="ht")
                nc.vector.tensor_scalar(
                    out=ht[:],
                    in0=tt_[:],
                    scalar1=float(2.0**23),
                    scalar2=0.0,
                    op0=Alu.subtract,
                    op1=Alu.add,
                    accum_out=partial[:, t : t + 1],
                )
                r1t = rlp.tile([128, SSH], dt.float32, tag="r1t")
                nc.vector.tensor_tensor_reduce(
                    out=r1t[:],
                    in0=rt[:],
                    in1=ht[:],
                    scale=1.0,
                    scalar=0.0,
                    op0=Alu.subtract,
                    op1=Alu.add,
                    accum_out=partial[:, JT + t : JT + t + 1],
                )

            cc_in = drp.tile([128, 2 * JT], dt.float32)
            cc_out = drp.tile([128, 2 * JT], dt.float32)
            nc.sync.dma_start(cc_in[:], partial[:])
            nc.gpsimd.collective_compute(
                "AllReduce",
                Alu.add,
                replica_groups=[list(range(N_CORES))],
                ins=[cc_in.opt()],
                outs=[cc_out.opt()],
            )
            hr = pp.tile([128, 2 * JT], dt.float32, tag="hr")
            nc.sync.dma_start(hr[:], cc_out[:])
            nc.vector.tensor_tensor(
                out=scores[:], in0=hr[:, :JT], in1=hr[:, JT:], op=Alu.add
            )
            nc.vector.tensor_scalar(
                out=scores[:],
                in0=scores[:],
                scalar1=float(2.0**-10),
                scalar2=None,
                op0=Alu.mult,
            )

            psC = psp.tile([DW, SCH], dt.float32, tag="ps0", name="psC_d")
            thr_f = _radix_select_threshold(
                nc, tc, pp, psC[:], ones128[:], bass_isa, mybir, scores, K, JT
            )
            nc.vector.tensor_scalar(
                out=mask[:],
                in0=scores[:],
                scalar1=thr_f[:],
                scalar2=None,
                op0=Alu.is_ge,
            )

            for c in range(NSCH):
                psums = [
                    psp.tile([DW, SCH], dt.float32, tag=f"ps{d}", name=f"ps_c{c}_d{d}")
                    for d in range(DT_)
                ]
                for t in range(JT):
                    xtile = xtp.tile([128, SCH], mmdt)
                    nc.sync.dma_start(
                        xtile[:], xt[t * 128 : (t + 1) * 128, c * SCH : (c + 1) * SCH]
                    )
                    wtile = wtp.tile([128, DSH], mmdt)
                    nc.sync.dma_start(wtile[:], wt[t * 128 : (t + 1) * 128, :])
                    wmt = wmp.tile([128, DSH], mmdt)
                    nc.vector.tensor_scalar(
                        out=wmt[:],
                        in0=wtile[:],
                        scalar1=mask[:, t : t + 1],
                        scalar2=None,
                        op0=Alu.mult,
                    )
                    for d in range(DT_):
                        nc.tensor.matmul(
                            psums[d][:],
                            lhsT=wmt[:, d * DW : (d + 1) * DW],
                            rhs=xtile[:],
                            start=(t == 0),
                            stop=(t == JT - 1),
                        )
                for d in range(DT_):
                    ot = otp.tile([DW, SCH], dt.float32)
                    nc.scalar.copy(ot[:], psums[d][:])
                    nc.sync.dma_start(
                        outT[d * DW : (d + 1) * DW, c * SCH : (c + 1) * SCH], ot[:]
                    )

    nc.compile()
    return nc


def _get_program(cfg):
    key = (cfg["name"], cfg.get("mm_dtype", "bf16"))
    if key not in _cache:
        if cfg.get("mode", "sparse") == "sparse":
            _cache[key] = _build_sparse(cfg)
        else:
            _cache[key] = _build_dense(cfg)
    return _cache[key]


def _stage_inputs(x, W, cfg):
    """Host-side sharding/layout. Returns per-core in_maps."""
    import ml_dtypes

    DFF = cfg["dff"]
    S = cfg["s"]
    D = cfg["d"]
    DSH = D // N_CORES
    SSH = S // N_CORES

    x2d = np.ascontiguousarray(np.asarray(x, dtype=np.float32).reshape(S, DFF))
    Wf = np.asarray(W, dtype=np.float32)

    xT = np.ascontiguousarray(x2d.T)          # [DFF, S]
    WT = np.ascontiguousarray(Wf.T)           # [DFF, D]

    if cfg.get("mm_dtype", "bf16") == "f32":
        xT_mm = xT
        WT_mm = WT
    else:
        xT_mm = xT.astype(ml_dtypes.bfloat16)
        WT_mm = WT.astype(ml_dtypes.bfloat16)

    in_maps = []
    for c in range(N_CORES):
        in_maps.append(
            {
                "xs": np.ascontiguousarray(xT[:, c * SSH : (c + 1) * SSH]),
                "xt": xT_mm,
                "wt": np.ascontiguousarray(WT_mm[:, c * DSH : (c + 1) * DSH]),
            }
        )
    return in_maps


def run_cfg(x, W, cfg, trace=False, trace_kwargs=None):
    """Run the kernel for a given cfg; returns (out, BassKernelResults)."""
    from concourse.bass_utils import run_bass_kernel_spmd

    S, D = cfg["s"], cfg["d"]
    DSH = D // N_CORES
    nc = _get_program(cfg)
    in_maps = _stage_inputs(x, W, cfg)
    res = run_bass_kernel_spmd(
        nc,
        in_maps,
        core_ids=list(range(N_CORES)),
        trace=trace,
        **(trace_kwargs or {}),
    )
    outT = np.concatenate([res.results[c]["outT"] for c in range(N_CORES)], axis=0)
    out = np.ascontiguousarray(outT.T).reshape(1, S, D).astype(np.float32)
    return out, res


def kernel(x, W):
    out, _ = run_cfg(x, W, FULL_CFG)
    return out


# revision 20
# speedup vs baseline: 1.0739x; 1.0739x over previous
"""Trainium2 Bass kernel for nn_CustomMLPLayer_13408887898971 (topk_masking).

Computes (matching reference.py):
    scores = sum_s relu(x[0,s,:])          # [d_ff]
    idx    = top_k(scores, K)              # K = 4403
    out    = x[..., idx] @ W[:, idx].T     # [1, S, d_model]

Strategy (8 NeuronCores, tensor-parallel over d_model), sparse mode:
  - host: transpose x and W to j-major (contraction on partitions),
    shard W.T by d_model columns (512 per core); x.T replicated (bf16
    for the GEMM, f32 token-shard for the exact score reduction).
  - device, per core:
      A: partial scores over this core's 256-token shard with a two-limb
         (integer part + fractional residue, scaled by 1024) split so the
         cross-core sum is exact to ~fp64 (the top-K boundary gap is only
         a few f32 ULP).
      B: AllReduce partial scores across the 8 cores (88KB)
      C: exact K-th largest via radix-16 search on the f32 bit pattern
      C2: compacted index list of the selected j's via gpsimd
          sparse_gather (stream compaction), replicated into the
          16-partition-wrapped int16 layout dma_gather wants
      D: dma_gather the K (padded to 4480) selected rows of x.T and W.T
         from HBM into SBUF (bf16) and run the compacted dense GEMM
         x_sel.T @ W_sel -> out.T shard; contraction is 4480 instead of
         11008 and runs at bf16 PE rate (1 cy/row) instead of fp32.
  - host: concat per-core [512, 2048] out.T shards, transpose.
"""

import numpy as np

N_CORES = 8

FULL_CFG = dict(
    dff=11008,
    s=2048,
    d=4096,
    k=4403,
    name="sparse",
    mode="sparse",
)

DENSE_CFG = dict(
    dff=11008,
    s=2048,
    d=4096,
    k=4403,
    name="dense",
    mode="dense",
    mm_dtype="bf16",
)

_cache = {}


def _radix_select_threshold(nc, tc, pp, psum_red, ones128, bass_isa, mybir, scores, K, JT):
    """Exact K-th largest score via radix-16 search on the f32 bit pattern.

    scores: [128, JT] f32, non-negative. Returns thr_f [128, 1] f32 tile
    (same value in every partition) with count(scores >= thr_f) >= K and
    count(scores >= next ulp) < K.

    The DVE ALU evaluates int32 tensor ops in f32 arithmetic, so bit-space
    increments below ULP(thr_bits ~ 2^30) = 128 round away. The int-bit
    stage resolves bits 7..30 (increments are multiples of 128 -> exact in
    f32); the low 7 bits are resolved in float space with exact ULP steps.

    The per-round cross-partition count reduction runs on the PE (ones
    matmul into `psum_red`: every output partition holds the column sum)
    instead of gpsimd partition_all_reduce - far lower latency. Candidate
    vectors are built 15-at-a-time in single DVE ops.
    """
    dt = mybir.dt
    Alu = mybir.AluOpType

    thr = pp.tile([128, 1], dt.int32, tag="thr")
    thrv = pp.tile([128, 1], dt.float32, tag="thrv")
    cands = pp.tile([128, 15], dt.int32, tag="cands")
    candf = pp.tile([128, 15], dt.float32, tag="candf")
    ge_scr = pp.tile([128, JT], dt.float32, tag="ge_scr")
    cnts = pp.tile([128, 15], dt.float32, tag="cnts")
    sel = pp.tile([128, 15], dt.float32, tag="sel")
    digf = pp.tile([128, 1], dt.float32, tag="digf")
    digi = pp.tile([128, 1], dt.int32, tag="digi")
    thr_f = pp.tile([128, 1], dt.float32, tag="thr_f")
    ulp = pp.tile([128, 1], dt.float32, tag="ulp")
    ulpm = pp.tile([128, 1], dt.float32, tag="ulpm")
    step = pp.tile([128, 1], dt.float32, tag="step")
    # rowi[:, r-1] = r (int32, every partition); rowv = same as f32
    rowi = pp.tile([128, 15], dt.int32, tag="rowi")
    rowv = pp.tile([128, 15], dt.float32, tag="rowv")
    nc.gpsimd.iota(rowi[:], pattern=[[1, 15]], base=1, channel_multiplier=0)
    nc.vector.tensor_copy(out=rowv[:], in_=rowi[:])

    nc.vector.memset(thrv[:], 0)

    def count_round(ncand, upd):
        """Counts candf[:, 0..ncand-1], picks the digit into digf, updates."""
        for r in range(ncand):
            nc.vector.tensor_scalar(
                out=ge_scr[:],
                in0=scores[:],
                scalar1=candf[:, r : r + 1],
                scalar2=0.0,
                op0=Alu.is_ge,
                op1=Alu.add,
                accum_out=cnts[:, r : r + 1],
            )
        nc.tensor.matmul(
            psum_red[:, :ncand],
            lhsT=ones128[:],
            rhs=cnts[:, :ncand],
            start=True,
            stop=True,
        )
        nc.vector.tensor_scalar(
            out=sel[:, :ncand],
            in0=psum_red[:, :ncand],
            scalar1=float(K),
            scalar2=0.0,
            op0=Alu.is_ge,
            op1=Alu.add,
            accum_out=digf[:],
        )
        upd()

    # --- int-bit stage: bits 7..30, radix 16 ---
    # The threshold bit pattern is carried as an f32 NUMERIC value thrv
    # (exact: always a multiple of 2^7 below 2^31); cands casts it to the
    # int32 bit pattern for the bitcast-compare.
    for shift in (27, 23, 19, 15, 11, 7):
        # cands[:, r-1] = thrv + (r << shift)
        nc.vector.tensor_scalar(
            out=cands[:],
            in0=rowi[:],
            scalar1=float(1 << shift),
            scalar2=thrv[:],
            op0=Alu.mult,
            op1=Alu.add,
        )
        nc.vector.tensor_scalar(
            out=candf[:],
            in0=cands[:].bitcast(dt.float32),
            scalar1=0.0,
            scalar2=None,
            op0=Alu.add,
        )

        def upd_int(shift=shift):
            nc.vector.tensor_scalar(
                out=thrv[:],
                in0=digf[:],
                scalar1=float(1 << shift),
                scalar2=thrv[:],
                op0=Alu.mult,
                op1=Alu.add,
            )

        count_round(15, upd_int)

    nc.vector.tensor_scalar(
        out=thr[:], in0=thrv[:], scalar1=0.0, scalar2=None, op0=Alu.add
    )

    # --- float stage: low 7 bits with exact ULP steps ---
    nc.vector.tensor_scalar(
        out=digi[:], in0=thr[:], scalar1=128, scalar2=None, op0=Alu.add
    )
    nc.vector.tensor_tensor(
        out=ulp[:],
        in0=digi[:].bitcast(dt.float32),
        in1=thr[:].bitcast(dt.float32),
        op=Alu.subtract,
    )
    nc.vector.tensor_scalar(
        out=ulp[:], in0=ulp[:], scalar1=1.0 / 128.0, scalar2=None, op0=Alu.mult
    )
    nc.vector.tensor_scalar(
        out=thr_f[:],
        in0=thr[:].bitcast(dt.float32),
        scalar1=0.0,
        scalar2=None,
        op0=Alu.add,
    )

    for mult_, ncand in ((16, 7), (1, 15)):
        # candf[:, r-1] = thr_f + (r * mult_) * ulp
        nc.vector.tensor_scalar(
            out=ulpm[:],
            in0=ulp[:],
            scalar1=float(mult_),
            scalar2=None,
            op0=Alu.mult,
        )
        nc.vector.tensor_scalar(
            out=candf[:, :ncand],
            in0=rowv[:, :ncand],
            scalar1=ulpm[:],
            scalar2=thr_f[:],
            op0=Alu.mult,
            op1=Alu.add,
        )

        def upd_f(mult_=mult_):
            nc.vector.tensor_scalar(
                out=digf[:],
                in0=digf[:],
                scalar1=float(mult_),
                scalar2=None,
                op0=Alu.mult,
            )
            nc.vector.tensor_tensor(out=step[:], in0=digf[:], in1=ulp[:], op=Alu.mult)
            nc.vector.tensor_tensor(
                out=thr_f[:], in0=thr_f[:], in1=step[:], op=Alu.add
            )

        count_round(ncand, upd_f)

    return thr_f


def _build_sparse(cfg):
    """Sparse top-K gather + compacted bf16 GEMM program."""
    from concourse import bacc, tile
    import concourse.bass as bass
    import concourse.mybir as mybir
    import concourse.bass_isa as bass_isa

    dt = mybir.dt
    Alu = mybir.AluOpType
    Act = mybir.ActivationFunctionType

    DFF = cfg["dff"]          # 11008
    S = cfg["s"]              # 2048
    D = cfg["d"]              # 4096
    K = cfg["k"]              # 4403
    DSH = D // N_CORES        # 512
    SSH = S // N_CORES        # 256
    JT = DFF // 128           # 86
    NKT = 35                  # k tiles (4480 slots >= K)
    NSLOT = NKT * 128         # 4480
    IDXW = NSLOT // 16        # 280 idx columns (16-wrapped)
    NXCH = 5                  # gather chunks
    KTC = NKT // NXCH         # 7 k-tiles per chunk
    CIDX = KTC * 128 // 16    # 56 idx cols per chunk
    CSLOT = KTC * 128         # 896 slots per chunk
    SGW = DFF // 16           # 688 sparse-gather input cols
    SCH = 512                 # matmul moving dim
    NCH = S // SCH            # 4 s-chunks
    DT_ = DSH // 128          # 4 d tiles

    nc = bacc.Bacc(
        "TRN2", target_bir_lowering=False, debug=False, num_devices=N_CORES
    )

    ROW = S + DSH              # 2560 bf16 = 5KB combined x|w row
    xs = nc.dram_tensor("xs", [DFF, SSH], dt.float32, kind="ExternalInput").ap()
    xw = nc.dram_tensor("xw", [DFF, ROW], dt.bfloat16, kind="ExternalInput").ap()
    outT = nc.dram_tensor("outT", [DSH, S], dt.float32, kind="ExternalOutput").ap()

    with tile.TileContext(nc) as tc:
        with (
            tc.tile_pool(name="persist", bufs=1) as pp,
            tc.tile_pool(name="xs_p", bufs=3) as xsp,
            tc.tile_pool(name="relu_p", bufs=2) as rlp,
            tc.tile_pool(name="out_p", bufs=2) as otp,
            tc.tile_pool(name="psum", bufs=1, space="PSUM") as psp,
            tc.tile_pool(name="dram", bufs=1, space="DRAM") as drp,
        ):
            # Q7 scratch limits sparse_gather to in+out <= ~44KB, i.e.
            # [16, 344] -> [16, 344] per call. Compact the 11008 slots in two
            # 5504-slot halves, then concatenate the two lists in DRAM: list
            # A at offset 0, list B at element offset nA (register-offset
            # DMA). Ascending writes make A's junk tail disappear under B's
            # copy, and B's tail is masked to -1 so slots [K', 4480) read
            # back as -1 (ignored by dma_gather).
            HT = JT // 2              # 43 j-tiles per half
            SGH = 16 * HT * 8 // 16   # 344 = free cols per half ([16, 344])
            HSLOT = 16 * SGH          # 5504 slots per half

            partial = pp.tile([128, 2 * JT], dt.float32, tag="partial")
            scores = pp.tile([128, JT], dt.float32, tag="scores")
            sc16 = [
                pp.tile([16, SGH], dt.float32, tag=f"sc16_{h}", name=f"sc16_{h}")
                for h in range(2)
            ]
            iotaf1 = pp.tile([16, SGH], dt.float32, tag="iotaf1")
            ioti = pp.tile([16, SGH], dt.int32, tag="ioti")
            sgout = [
                pp.tile([16, SGH], dt.float32, tag=f"sgout_{h}", name=f"sgout_{h}")
                for h in range(2)
            ]
            nf = [
                pp.tile([1, 1], dt.uint32, tag=f"nf_{h}", name=f"nf_{h}")
                for h in range(2)
            ]
            slot16 = pp.tile([16, SGH], dt.float32, tag="slot16")
            nb16 = pp.tile([16, 1], dt.uint32, tag="nb16")
            nb16f = pp.tile([16, 1], dt.float32, tag="nb16f")
            idxsff = pp.tile([128, IDXW], dt.float32, tag="idxsff")
            idxs16 = pp.tile([128, IDXW], dt.int16, tag="idxs16")
            xg = pp.tile([128, NKT, ROW], dt.bfloat16, tag="xg")

            # slot-number table for tail masking: slot16[p, f] = 16f + p
            nc.gpsimd.iota(
                ioti[:], pattern=[[16, SGH]], base=0, channel_multiplier=1
            )
            nc.vector.tensor_copy(out=slot16[:], in_=ioti[:])
            # pad slots (>= K') of the last k-tile stay zero => zero
            # contribution in the GEMM, no masking needed anywhere.
            nc.vector.memset(xg[:, NKT - 1 : NKT, :], 0)
            ones128 = pp.tile([128, 128], dt.float32, tag="ones128")
            nc.vector.memset(ones128[:], 1.0)

            # ---- phase A: partial scores over this core's token shard ----
            # Two-limb trick on r = relu(x)*1024: integer part h sums EXACTLY
            # in f32 (partials are integers < 2^24); residue |r1| <= 0.5 sums
            # with ~1e-6 relative noise. h = (r + 2^23) - 2^23 (RNE round).
            for t in range(JT):
                st = xsp.tile([128, SSH], dt.float32)
                nc.sync.dma_start(st[:], xs[t * 128 : (t + 1) * 128, :])
                rt = rlp.tile([128, SSH], dt.float32, tag="rt")
                nc.scalar.activation(rt[:], st[:], Act.Relu, scale=1024.0)
                tmpt = rlp.tile([128, SSH], dt.float32, tag="tmpt")
                nc.vector.tensor_scalar(
                    out=tmpt[:],
                    in0=rt[:],
                    scalar1=float(2.0**23),
                    scalar2=None,
                    op0=Alu.add,
                )
                ht = rlp.tile([128, SSH], dt.float32, tag="ht")
                nc.vector.tensor_scalar(
                    out=ht[:],
                    in0=tmpt[:],
                    scalar1=float(2.0**23),
                    scalar2=0.0,
                    op0=Alu.subtract,
                    op1=Alu.add,
                    accum_out=partial[:, t : t + 1],
                )
                r1t = rlp.tile([128, SSH], dt.float32, tag="r1t")
                nc.vector.tensor_tensor(
                    out=r1t[:], in0=rt[:], in1=ht[:], op=Alu.subtract
                )
                nc.vector.tensor_reduce(
                    out=partial[:, JT + t : JT + t + 1],
                    in_=r1t[:],
                    axis=mybir.AxisListType.X,
                    op=Alu.add,
                )

            # ---- phase B: AllReduce partial sums across cores ----
            cc_in = drp.tile([128, 2 * JT], dt.float32)
            cc_out = drp.tile([128, 2 * JT], dt.float32)
            nc.sync.dma_start(cc_in[:], partial[:])
            nc.gpsimd.collective_compute(
                "AllReduce",
                Alu.add,
                replica_groups=[list(range(N_CORES))],
                ins=[cc_in.opt()],
                outs=[cc_out.opt()],
            )
            hr = pp.tile([128, 2 * JT], dt.float32, tag="hr")
            nc.sync.dma_start(hr[:], cc_out[:])
            nc.vector.tensor_tensor(
                out=scores[:], in0=hr[:, :JT], in1=hr[:, JT:], op=Alu.add
            )
            nc.vector.tensor_scalar(
                out=scores[:],
                in0=scores[:],
                scalar1=float(2.0**-10),
                scalar2=None,
                op0=Alu.mult,
            )

            # ---- phase C: K-th largest threshold ----
            psC = psp.tile([128, SCH], dt.float32, tag="ps0", name="psC")
            thr_f = _radix_select_threshold(
                nc, tc, pp, psC[:], ones128[:], bass_isa, mybir, scores, K, JT
            )

            # ---- phase C2: compacted index list (two halves) ----
            # sc16[h][p, 43a + t] = scores[16a + p, 43h + t], i.e. slot
            # (p, f) of half h holds the score of j = 5504h + 16a + 128t + p,
            # matching iotaf1[h] (which holds j+1).
            for h in range(2):
                for a in range(8):
                    nc.sync.dma_start(
                        sc16[h][:, HT * a : HT * (a + 1)],
                        scores[16 * a : 16 * (a + 1), HT * h : HT * (h + 1)],
                    )

            cat = drp.tile([1, 2 * HSLOT], dt.float32)

            for h in range(2):
                # Half h covers j in [5504h, 5504(h+1)); slot (p, f=a*43+b)
                # holds j = 5504h + 16a + 128b + p, matching the 8
                # per-16-partition-block column copies above. iotaf1 = j+1.
                nc.gpsimd.iota(
                    ioti[:],
                    pattern=[[16, 8], [128, HT]],
                    base=HSLOT * h + 1,
                    channel_multiplier=1,
                )
                nc.vector.tensor_copy(out=iotaf1[:], in_=ioti[:])
                # selj (in place over sc16) = sel ? j : -1 = ge*(j+1) - 1
                nc.vector.tensor_scalar(
                    out=sc16[h][:],
                    in0=sc16[h][:],
                    scalar1=thr_f[0:16, :],
                    scalar2=None,
                    op0=Alu.is_ge,
                )
                nc.vector.tensor_tensor(
                    out=sc16[h][:], in0=sc16[h][:], in1=iotaf1[:], op=Alu.mult
                )
                nc.vector.tensor_scalar(
                    out=sc16[h][:],
                    in0=sc16[h][:],
                    scalar1=1.0,
                    scalar2=None,
                    op0=Alu.subtract,
                )
                nc.gpsimd.sparse_gather(sgout[h][:], sc16[h][:], num_found=nf[h][:])

            # mask list B's junk tail (hw writes arbitrary values past
            # num_found): slots >= nB -> -1.
            nc.vector.tensor_copy(out=nb16[0:1, :], in_=nf[1][:])
            for p in (1, 2, 4, 8):
                nc.sync.dma_start(nb16[p : 2 * p, :], nb16[0:p, :])
            nc.vector.tensor_copy(out=nb16f[:], in_=nb16[:])
            bmask = iotaf1  # reuse scratch
            nc.vector.tensor_scalar(
                out=bmask[:],
                in0=slot16[:],
                scalar1=nb16f[:],
                scalar2=None,
                op0=Alu.is_lt,
            )
            # sgoutB = valid ? sgoutB : -1 == bmask * (sgoutB + 1) - 1
            nc.vector.tensor_scalar(
                out=sgout[1][:],
                in0=sgout[1][:],
                scalar1=1.0,
                scalar2=None,
                op0=Alu.add,
            )
            nc.vector.tensor_tensor(
                out=sgout[1][:], in0=sgout[1][:], in1=bmask[:], op=Alu.mult
            )
            nc.vector.tensor_scalar(
                out=sgout[1][:],
                in0=sgout[1][:],
                scalar1=1.0,
                scalar2=None,
                op0=Alu.subtract,
            )

            # registers (Pool engine: same queue as the gpsimd DMAs below)
            nAr = nc.alloc_register(mybir.EngineType.Pool, "nAr")
            nfr = nc.alloc_register(mybir.EngineType.Pool, "nfr")
            wreg = nc.alloc_register(mybir.EngineType.Pool, "wreg")
            r4 = nc.alloc_register(mybir.EngineType.Pool, "r4")
            nc.gpsimd.reg_load(nAr, nf[0][:1, :1])
            nc.gpsimd.reg_load(nfr, nf[1][:1, :1])
            nc.gpsimd.reg_alu(nfr, nfr, nAr, Alu.add)
            nc.gpsimd.reg_alu(wreg, nfr, NSLOT, Alu.min)
            nc.gpsimd.reg_alu(r4, wreg, (NXCH - 1) * CSLOT, Alu.subtract)

            # concatenate in DRAM (linear slot order; de-wrap [16, F] via
            # [(1,16),(16,F)] APs): A at 0, B at element offset nA.
            cat_t = cat[:].tensor
            catA = bass.AP(cat_t, 0, [[1, 16], [16, SGH]])
            nc.gpsimd.dma_start(out=catA, in_=sgout[0][:])
            catB = bass.AP(
                cat_t, nAr, [[1, 16], [16, SGH]], dep_tracking_offset=0
            )
            nc.gpsimd.dma_start(out=catB, in_=sgout[1][:])
            # read back the first 4480 slots, re-wrapped to [16, 280]
            catR = bass.AP(cat_t, 0, [[1, 16], [16, IDXW]])
            for b in range(8):
                nc.sync.dma_start(out=idxsff[16 * b : 16 * (b + 1), :], in_=catR)
            nc.vector.tensor_copy(out=idxs16[:], in_=idxsff[:])

            # ---- phase D: combined x|w row gather + compacted GEMM ----
            # One 5KB descriptor per selected row covers both operands --
            # halves the (serial, Q7-bound) SWDGE descriptor generation
            # vs separate x and W gathers.
            for i in range(NXCH):
                ri = CSLOT if i < NXCH - 1 else r4
                nc.gpsimd.dma_gather(
                    out_ap=xg[:, KTC * i : KTC * (i + 1), :],
                    in_ap=xw[:],
                    idxs_ap=idxs16[:, CIDX * i : CIDX * (i + 1)],
                    num_idxs=CSLOT,
                    num_idxs_reg=ri,
                    elem_size=ROW,
                )

            for half in range(2):
                cs = (2 * half, 2 * half + 1)
                psums = [
                    psp.tile([128, SCH], dt.float32, tag=f"ps{j}", name=f"ps_h{half}_{j}")
                    for j in range(8)
                ]
                for g in range(NKT):
                    for ci, c in enumerate(cs):
                        rhs = xg[:, g : g + 1, c * SCH : (c + 1) * SCH].squeeze(1)
                        for d in range(DT_):
                            nc.tensor.matmul(
                                psums[ci * DT_ + d][:],
                                lhsT=xg[
                                    :, g : g + 1, S + d * 128 : S + (d + 1) * 128
                                ].squeeze(1),
                                rhs=rhs,
                                start=(g == 0),
                                stop=(g == NKT - 1),
                            )
                for ci, c in enumerate(cs):
                    for d in range(DT_):
                        ot = otp.tile([128, SCH], dt.float32)
                        nc.scalar.copy(ot[:], psums[ci * DT_ + d][:])
                        nc.sync.dma_start(
                            outT[d * 128 : (d + 1) * 128, c * SCH : (c + 1) * SCH],
                            ot[:],
                        )

    nc.compile()
    return nc


def _build_dense(cfg):
    """Masked dense GEMM program (fallback / ablation)."""
    from concourse import bacc, tile
    import concourse.bass as bass
    import concourse.mybir as mybir
    import concourse.bass_isa as bass_isa

    dt = mybir.dt
    Alu = mybir.AluOpType
    Act = mybir.ActivationFunctionType

    DFF = cfg["dff"]
    S = cfg["s"]
    D = cfg["d"]
    K = cfg["k"]
    DSH = D // N_CORES
    SSH = S // N_CORES
    JT = DFF // 128
    SCH = min(512, S)
    NSCH = S // SCH
    DT_ = max(1, DSH // 128)
    DW = min(128, DSH)

    mmdt = dt.float32 if cfg.get("mm_dtype", "bf16") == "f32" else dt.bfloat16

    nc = bacc.Bacc(
        "TRN2", target_bir_lowering=False, debug=False, num_devices=N_CORES
    )

    xs = nc.dram_tensor("xs", [DFF, SSH], dt.float32, kind="ExternalInput").ap()
    xt = nc.dram_tensor("xt", [DFF, S], mmdt, kind="ExternalInput").ap()
    wt = nc.dram_tensor("wt", [DFF, DSH], mmdt, kind="ExternalInput").ap()
    outT = nc.dram_tensor("outT", [DSH, S], dt.float32, kind="ExternalOutput").ap()

    with tile.TileContext(nc) as tc:
        with (
            tc.tile_pool(name="persist", bufs=1) as pp,
            tc.tile_pool(name="xs_p", bufs=3) as xsp,
            tc.tile_pool(name="relu_p", bufs=2) as rlp,
            tc.tile_pool(name="xt_p", bufs=4) as xtp,
            tc.tile_pool(name="wt_p", bufs=4) as wtp,
            tc.tile_pool(name="wm_p", bufs=4) as wmp,
            tc.tile_pool(name="out_p", bufs=3) as otp,
            tc.tile_pool(name="psum", bufs=2, space="PSUM") as psp,
            tc.tile_pool(name="dram", bufs=1, space="DRAM") as drp,
        ):
            partial = pp.tile([128, 2 * JT], dt.float32, tag="partial")
            scores = pp.tile([128, JT], dt.float32, tag="scores")
            mask = pp.tile([128, JT], dt.float32, tag="mask")
            c23 = pp.tile([128, 1], dt.float32, tag="c23")
            nc.vector.memset(c23[:], float(2.0**23))
            ones128 = pp.tile([128, 128], dt.float32, tag="ones128")
            nc.vector.memset(ones128[:], 1.0)

            for t in range(JT):
                st = xsp.tile([128, SSH], dt.float32)
                nc.sync.dma_start(st[:], xs[t * 128 : (t + 1) * 128, :])
                rt = rlp.tile([128, SSH], dt.float32, tag="rt")
                nc.scalar.activation(rt[:], st[:], Act.Relu, scale=1024.0)
                tt_ = rlp.tile([128, SSH], dt.float32, tag="tt")
                nc.scalar.activation(tt_[:], rt[:], Act.Identity, bias=c23[:])
                ht = rlp.tile([128, SSH], dt.float32, tag="ht")
                nc.vector.tensor_scalar(
                    out=ht[:],
                    in0=tt_[:],
                    scalar1=float(2.0**23),
                    scalar2=0.0,
                    op0=Alu.subtract,
                    op1=Alu.add,
                    accum_out=partial[:, t : t + 1],
                )
                r1t = rlp.tile([128, SSH], dt.float32, tag="r1t")
                nc.vector.tensor_tensor_reduce(
                    out=r1t[:],
                    in0=rt[:],
                    in1=ht[:],
                    scale=1.0,
                    scalar=0.0,
                    op0=Alu.subtract,
                    op1=Alu.add,
                    accum_out=partial[:, JT + t : JT + t + 1],
                )

            cc_in = drp.tile([128, 2 * JT], dt.float32)
            cc_out = drp.tile([128, 2 * JT], dt.float32)
            nc.sync.dma_start(cc_in[:], partial[:])
            nc.gpsimd.collective_compute(
                "AllReduce",
                Alu.add,
                replica_groups=[list(range(N_CORES))],
                ins=[cc_in.opt()],
                outs=[cc_out.opt()],
            )
            hr = pp.tile([128, 2 * JT], dt.float32, tag="hr")
            nc.sync.dma_start(hr[:], cc_out[:])
            nc.vector.tensor_tensor(
                out=scores[:], in0=hr[:, :JT], in1=hr[:, JT:], op=Alu.add
            )
            nc.vector.tensor_scalar(
                out=scores[:],
                in0=scores[:],
                scalar1=float(2.0**-10),
                scalar2=None,
                op0=Alu.mult,
            )

            psC = psp.tile([DW, SCH], dt.float32, tag="ps0", name="psC_d")
            thr_f = _radix_select_threshold(
                nc, tc, pp, psC[:], ones128[:], bass_isa, mybir, scores, K, JT
            )
            nc.vector.tensor_scalar(
                out=mask[:],
                in0=scores[:],
                scalar1=thr_f[:],
                scalar2=None,
                op0=Alu.is_ge,
            )

            for c in range(NSCH):
                psums = [
                    psp.tile([DW, SCH], dt.float32, tag=f"ps{d}", name=f"ps_c{c}_d{d}")
                    for d in range(DT_)
                ]
                for t in range(JT):
                    xtile = xtp.tile([128, SCH], mmdt)
                    nc.sync.dma_start(
                        xtile[:], xt[t * 128 : (t + 1) * 128, c * SCH : (c + 1) * SCH]
                    )
                    wtile = wtp.tile([128, DSH], mmdt)
                    nc.sync.dma_start(wtile[:], wt[t * 128 : (t + 1) * 128, :])
                    wmt = wmp.tile([128, DSH], mmdt)
                    nc.vector.tensor_scalar(
                        out=wmt[:],
                        in0=wtile[:],
                        scalar1=mask[:, t : t + 1],
                        scalar2=None,
                        op0=Alu.mult,
                    )
                    for d in range(DT_):
                        nc.tensor.matmul(
                            psums[d][:],
                            lhsT=wmt[:, d * DW : (d + 1) * DW],
                            rhs=xtile[:],
                            start=(t == 0),
                            stop=(t == JT - 1),
                        )
                for d in range(DT_):
                    ot = otp.tile([DW, SCH], dt.float32)
                    nc.scalar.copy(ot[:], psums[d][:])
                    nc.sync.dma_start(
                        outT[d * DW : (d + 1) * DW, c * SCH : (c + 1) * SCH], ot[:]
                    )

    nc.compile()
    return nc


def _get_program(cfg):
    key = (cfg["name"], cfg.get("mm_dtype", "bf16"))
    if key not in _cache:
        if cfg.get("mode", "sparse") == "sparse":
            _cache[key] = _build_sparse(cfg)
        else:
            _cache[key] = _build_dense(cfg)
    return _cache[key]


def _stage_inputs(x, W, cfg):
    """Host-side sharding/layout. Returns per-core in_maps."""
    import ml_dtypes

    DFF = cfg["dff"]
    S = cfg["s"]
    D = cfg["d"]
    DSH = D // N_CORES
    SSH = S // N_CORES

    x2d = np.ascontiguousarray(np.asarray(x, dtype=np.float32).reshape(S, DFF))
    Wf = np.asarray(W, dtype=np.float32)

    xT = np.ascontiguousarray(x2d.T)          # [DFF, S]
    WT = np.ascontiguousarray(Wf.T)           # [DFF, D]

    if cfg.get("mm_dtype", "bf16") == "f32":
        xT_mm = xT
        WT_mm = WT
    else:
        xT_mm = xT.astype(ml_dtypes.bfloat16)
        WT_mm = WT.astype(ml_dtypes.bfloat16)

    in_maps = []
    for c in range(N_CORES):
        if cfg.get("mode", "sparse") == "sparse":
            xwc = np.concatenate(
                [xT_mm, WT_mm[:, c * DSH : (c + 1) * DSH]], axis=1
            )
            in_maps.append(
                {
                    "xs": np.ascontiguousarray(xT[:, c * SSH : (c + 1) * SSH]),
                    "xw": np.ascontiguousarray(xwc),
                }
            )
        else:
            in_maps.append(
                {
                    "xs": np.ascontiguousarray(xT[:, c * SSH : (c + 1) * SSH]),
                    "xt": xT_mm,
                    "wt": np.ascontiguousarray(WT_mm[:, c * DSH : (c + 1) * DSH]),
                }
            )
    return in_maps


def run_cfg(x, W, cfg, trace=False, trace_kwargs=None):
    """Run the kernel for a given cfg; returns (out, BassKernelResults)."""
    from concourse.bass_utils import run_bass_kernel_spmd

    S, D = cfg["s"], cfg["d"]
    DSH = D // N_CORES
    nc = _get_program(cfg)
    in_maps = _stage_inputs(x, W, cfg)
    res = run_bass_kernel_spmd(
        nc,
        in_maps,
        core_ids=list(range(N_CORES)),
        trace=trace,
        **(trace_kwargs or {}),
    )
    outT = np.concatenate([res.results[c]["outT"] for c in range(N_CORES)], axis=0)
    out = np.ascontiguousarray(outT.T).reshape(1, S, D).astype(np.float32)
    return out, res


def kernel(x, W):
    out, _ = run_cfg(x, W, FULL_CFG)
    return out
